# revision 8
# baseline (speedup 1.0000x reference)
"""Trainium2 Bass kernel for nn_BilinearScorer.

Reference computation (per full input):
    t = text @ W_text.T + b_text            # [B, H]
    v = t @ W_patch                         # [B, PD]
    scores[b, n] = patches[b, n, :] . v[b]  + t[b] . b_patch   # [B, N]

Strategy: data-parallel over batch B across 8 NeuronCores (4 batches/core).
The heavy op (patches . v) is HBM-bandwidth bound (64 MiB patches per core).
Per core:
  - the small projections run on the otherwise-idle TensorEngine:
    t^T from transposed W_text/text loads, then per-batch v rows
    (lhsT = t^T column), then ones-vector matmuls replicate v[b]/bias[b]
    across all 128 partitions (ScalarE copies PSUM->SBUF);
  - main loop: stream 4 MiB patch tiles [128, 8, 1024] via HWDGE DMA; one
    fused DVE scalar_tensor_tensor per 128-row block computes the dot
    product (accum_out); per-batch bias is added once on the [128, 32]
    score tile at the end.
Output is written as [BL, 128, 32] (partition-major) and transposed on host.
"""

import os
import sys

import numpy as np

_REPO = "/opt/trn_rl_repo"
if _REPO not in sys.path:
    sys.path.insert(0, _REPO)

B, N, PD, TD, H = 32, 4096, 1024, 768, 512
NCORES = 8
BL = B // NCORES          # batches per core
P = 128                   # partitions
NB = N // P               # 32 n-blocks of 128 rows
JPT = 8                   # n-blocks per DMA tile (4 MiB per DMA)
NT = NB // JPT            # DMA tiles per batch
HC = H // P               # h chunks
TC = TD // P              # text-dim chunks
PATCH_BUFS = 3

_NC_CACHE = {}
LAST_RESULTS = None       # BassKernelResults of the most recent kernel() call


def _build_nc():
    import concourse.bacc as bacc
    import concourse.bass as bass
    import concourse.mybir as mybir
    from concourse.tile import TileContext

    f32 = mybir.dt.float32
    mult = mybir.AluOpType.mult

    nc = bacc.Bacc("TRN2", target_bir_lowering=False, debug=False,
                   num_devices=NCORES)

    patches = nc.dram_tensor("patches", [BL, N, PD], f32, kind="ExternalInput")[:]
    text = nc.dram_tensor("text", [BL, TD], f32, kind="ExternalInput")[:]
    w_patch = nc.dram_tensor("w_patch", [H, PD], f32, kind="ExternalInput")[:]
    b_patch = nc.dram_tensor("b_patch", [H], f32, kind="ExternalInput")[:]
    w_text = nc.dram_tensor("w_text", [H, TD], f32, kind="ExternalInput")[:]
    b_text = nc.dram_tensor("b_text", [H], f32, kind="ExternalInput")[:]
    scores = nc.dram_tensor("scores", [BL, P, NB], f32, kind="ExternalOutput")[:]

    with TileContext(nc) as tc:
        with (
            tc.tile_pool(name="const", bufs=1) as const,
            tc.tile_pool(name="patch", bufs=PATCH_BUFS) as ppool,
            tc.tile_pool(name="psum", bufs=1, space=bass.MemorySpace.PSUM) as psum,
        ):
            # ---- small-weight loads ----
            # W_text^T tiles [td-chunk 128, H] (strided: 512B contiguous runs)
            wtT = []
            for c in range(TC):
                t_ = const.tile([P, H], f32, name=f"wtT{c}")
                nc.sync.dma_start(
                    out=t_[:], in_=w_text.rearrange("h (c p) -> c p h", p=P)[c]
                )
                wtT.append(t_)
            txT = []
            for c in range(TC):
                t_ = const.tile([P, BL], f32, name=f"txT{c}")
                nc.sync.dma_start(
                    out=t_[:], in_=text.rearrange("b (c p) -> c p b", p=P)[c]
                )
                txT.append(t_)
            bt_row = const.tile([1, H], f32, name="bt_row")
            nc.sync.dma_start(out=bt_row[:], in_=b_text[None, :])
            bp_sb = const.tile([P, HC], f32, name="bp_sb")
            nc.sync.dma_start(out=bp_sb[:], in_=b_patch.rearrange("(c p) -> p c", p=P))
            wp_sb = []
            for c in range(HC):
                t_ = const.tile([P, PD], f32, name=f"wp{c}")
                nc.sync.dma_start(
                    out=t_[:], in_=w_patch.rearrange("(c p) d -> c p d", p=P)[c]
                )
                wp_sb.append(t_)
            ones4 = const.tile([1, BL], f32, name="ones4")
            nc.vector.memset(ones4[:], 1.0)
            ones128 = const.tile([1, P], f32, name="ones128")
            nc.vector.memset(ones128[:], 1.0)

            # ---- t^T[h, b] per h-chunk (PE, b_text folded via K=1 matmul) ----
            tT_sb = []
            for c in range(HC):
                tT_ps = psum.tile([P, BL], f32, name=f"tT_ps{c}", tag="tT_ps")
                for k in range(TC):
                    nc.tensor.matmul(
                        tT_ps[:],
                        lhsT=wtT[k][:, c * P : (c + 1) * P],
                        rhs=txT[k][:],
                        start=(k == 0),
                        stop=False,
                    )
                nc.tensor.matmul(
                    tT_ps[:],
                    lhsT=bt_row[0:1, c * P : (c + 1) * P],
                    rhs=ones4[:],
                    start=False,
                    stop=True,
                )
                t_ = const.tile([P, BL], f32, name=f"tT{c}")
                nc.scalar.copy(out=t_[:], in_=tT_ps[:])
                tT_sb.append(t_)

            # ---- per-batch v rows + partition-broadcast (PE + ACT) ----
            vbc = []
            for b in range(BL):
                v_row = const.tile([1, PD], f32, name=f"v_row{b}")
                for half in range(PD // 512):
                    v_ps = psum.tile([1, 512], f32, name=f"v_ps{b}_{half}", tag="v_ps")
                    for c in range(HC):
                        nc.tensor.matmul(
                            v_ps[:],
                            lhsT=tT_sb[c][:, b : b + 1],
                            rhs=wp_sb[c][:, half * 512 : (half + 1) * 512],
                            start=(c == 0),
                            stop=(c == HC - 1),
                        )
                    nc.scalar.copy(
                        out=v_row[0:1, half * 512 : (half + 1) * 512], in_=v_ps[:]
                    )
                vb_sb = const.tile([P, PD], f32, name=f"vbc{b}")
                for half in range(PD // 512):
                    vb_ps = psum.tile(
                        [P, 512], f32, name=f"vb_ps{b}_{half}", tag="vb_ps", bufs=2
                    )
                    nc.tensor.matmul(
                        vb_ps[:],
                        lhsT=ones128[:],
                        rhs=v_row[0:1, half * 512 : (half + 1) * 512],
                        start=True,
                        stop=True,
                    )
                    nc.scalar.copy(
                        out=vb_sb[:, half * 512 : (half + 1) * 512], in_=vb_ps[:]
                    )
                vbc.append(vb_sb)

            # ---- per-batch bias rows + broadcast ----
            br_sb = const.tile([1, BL], f32, name="br_sb")
            for b in range(BL):
                br_ps = psum.tile([1, 1], f32, name=f"br_ps{b}", tag="br_ps")
                for c in range(HC):
                    nc.tensor.matmul(
                        br_ps[:],
                        lhsT=tT_sb[c][:, b : b + 1],
                        rhs=bp_sb[:, c : c + 1],
                        start=(c == 0),
                        stop=(c == HC - 1),
                    )
                nc.scalar.copy(out=br_sb[0:1, b : b + 1], in_=br_ps[:])
            bbc_ps = psum.tile([P, BL], f32, name="bbc_ps", tag="bbc_ps")
            nc.tensor.matmul(
                bbc_ps[:], lhsT=ones128[:], rhs=br_sb[:], start=True, stop=True
            )
            bbc = const.tile([P, BL], f32, name="bbc")
            nc.scalar.copy(out=bbc[:], in_=bbc_ps[:])

            # ---- main loop: scores[b, n] = bias[b] + patches[b, n, :] . v[b] ----
            prod = const.tile([P, PD], f32, name="prod")
            for b in range(BL):
                sc_sb = const.tile([P, NB], f32, name=f"sc{b}")
                pr = patches[b].rearrange("(t j p) d -> t p j d", p=P, j=JPT)
                for t in range(NT):
                    tile_ = ppool.tile([P, JPT, PD], f32, tag="ptile", name="ptile")
                    nc.sync.dma_start(out=tile_[:], in_=pr[t])
                    for j in range(JPT):
                        nc.vector.scalar_tensor_tensor(
                            out=prod[:],
                            in0=tile_[:, j, :],
                            scalar=1.0,
                            in1=vbc[b][:, :],
                            op0=mult,
                            op1=mult,
                            accum_out=sc_sb[:, t * JPT + j : t * JPT + j + 1],
                        )
                nc.vector.tensor_scalar_add(
                    out=sc_sb[:, :], in0=sc_sb[:, :], scalar1=bbc[:, b : b + 1]
                )
                nc.sync.dma_start(out=scores[b], in_=sc_sb[:])

    nc.compile()
    return nc


def _get_nc():
    if "nc" not in _NC_CACHE:
        _NC_CACHE["nc"] = _build_nc()
    return _NC_CACHE["nc"]


def _install_profile_shim():
    """Provide antenv.axon_hooks (NTFF profiling over axon) when absent.

    Replicates trn_agent_boot's ctypes hook against libaxon_pjrt.so so
    run_bass_kernel_spmd(trace=True) can capture device profiles."""
    import contextlib
    import ctypes
    import types

    try:
        from antenv.axon_hooks import get_axon_ntff_profile_hook  # noqa: F401
        return
    except ImportError:
        pass

    so_path = "/opt/axon/libaxon_pjrt.so"
    hook = None
    if os.path.exists(so_path):
        lib = ctypes.CDLL(so_path)
        if hasattr(lib, "axon_start_nrt_profile"):
            lib.axon_start_nrt_profile.argtypes = [
                ctypes.POINTER(ctypes.c_int64),
                ctypes.c_size_t,
            ]
            lib.axon_start_nrt_profile.restype = ctypes.c_int64
            lib.axon_stop_nrt_profile.argtypes = [ctypes.c_char_p]
            lib.axon_stop_nrt_profile.restype = ctypes.c_int64

            @contextlib.contextmanager
            def _hook(output_dir, device_ids):
                import jax

                jax.devices()
                if device_ids:
                    ids = (ctypes.c_int64 * len(device_ids))(*device_ids)
                    rc = lib.axon_start_nrt_profile(ids, len(device_ids))
                else:
                    rc = lib.axon_start_nrt_profile(None, 0)
                if rc != 0:
                    raise RuntimeError(f"axon_start_nrt_profile rc={rc}")
                try:
                    yield
                finally:
                    n = lib.axon_stop_nrt_profile(str(output_dir).encode())
                    print(f"ntff profile: {n} file(s) -> {output_dir}",
                          file=sys.stderr)

            hook = _hook

    mod = types.ModuleType("antenv.axon_hooks")
    mod.get_axon_ntff_profile_hook = lambda: hook
    mod.set_axon_ntff_profile_hook = lambda h: None
    sys.modules["antenv.axon_hooks"] = mod


def kernel(**inputs):
    from concourse.bass_utils import run_bass_kernel_spmd

    global LAST_RESULTS

    patches = np.ascontiguousarray(np.asarray(inputs["patches"], dtype=np.float32))
    text = np.ascontiguousarray(np.asarray(inputs["text"], dtype=np.float32))
    w_patch = np.ascontiguousarray(np.asarray(inputs["W_patch"], dtype=np.float32))
    b_patch = np.ascontiguousarray(np.asarray(inputs["b_patch"], dtype=np.float32))
    w_text = np.ascontiguousarray(np.asarray(inputs["W_text"], dtype=np.float32))
    b_text = np.ascontiguousarray(np.asarray(inputs["b_text"], dtype=np.float32))

    nc = _get_nc()
    in_maps = []
    for c in range(NCORES):
        in_maps.append(
            {
                "patches": patches[c * BL : (c + 1) * BL],
                "text": text[c * BL : (c + 1) * BL],
                "w_patch": w_patch,
                "b_patch": b_patch,
                "w_text": w_text,
                "b_text": b_text,
            }
        )

    trace = bool(int(os.environ.get("KERNEL_PROFILE", "0")))
    if trace:
        _install_profile_shim()
        import concourse.bass_utils as _bu

        _bu.upload_artifacts = lambda tmpdir: ""  # no artifact bucket here
    res = run_bass_kernel_spmd(
        nc, in_maps, core_ids=list(range(NCORES)), trace=trace
    )
    LAST_RESULTS = res

    out = np.concatenate(
        [
            np.transpose(res.results[c]["scores"], (0, 2, 1)).reshape(BL, N)
            for c in range(NCORES)
        ],
        axis=0,
    )
    return out


# revision 9
# speedup vs baseline: 2.1966x; 2.1966x over previous
"""Trainium2 Bass kernel for nn_BilinearScorer.

Reference computation (per full input):
    t = text @ W_text.T + b_text            # [B, H]
    v = t @ W_patch                         # [B, PD]
    scores[b, n] = patches[b, n, :] . v[b]  + t[b] . b_patch   # [B, N]

Strategy: data-parallel over batch B across 8 NeuronCores (4 batches/core).
The heavy op (patches . v) is HBM-bandwidth bound (64 MiB patches per core).
Per core:
  - the small projections run on the otherwise-idle TensorEngine:
    t^T from transposed W_text/text loads, then per-batch v rows
    (lhsT = t^T column), then ones-vector matmuls replicate v[b]/bias[b]
    across all 128 partitions (ScalarE copies PSUM->SBUF);
  - main loop: stream 4 MiB patch tiles [128, 8, 1024] via HWDGE DMA; one
    fused DVE scalar_tensor_tensor per 128-row block computes the dot
    product (accum_out); per-batch bias is added once on the [128, 32]
    score tile at the end.
Output is written as [BL, 128, 32] (partition-major) and transposed on host.
"""

import os
import sys

import numpy as np

_REPO = "/opt/trn_rl_repo"
if _REPO not in sys.path:
    sys.path.insert(0, _REPO)

B, N, PD, TD, H = 32, 4096, 1024, 768, 512
NCORES = 8
BL = B // NCORES          # batches per core
P = 128                   # partitions
NB = N // P               # 32 n-blocks of 128 rows
JPT = 4                   # n-blocks per DMA tile (2 MiB per DMA)
NT = NB // JPT            # DMA tiles per batch
HC = H // P               # h chunks
TC = TD // P              # text-dim chunks
PATCH_BUFS = 6

_NC_CACHE = {}
LAST_RESULTS = None       # BassKernelResults of the most recent kernel() call


def _build_nc():
    import concourse.bacc as bacc
    import concourse.bass as bass
    import concourse.mybir as mybir
    from concourse.tile import TileContext

    f32 = mybir.dt.float32
    mult = mybir.AluOpType.mult

    nc = bacc.Bacc("TRN2", target_bir_lowering=False, debug=False,
                   num_devices=NCORES)

    patches = nc.dram_tensor("patches", [BL, N, PD], f32, kind="ExternalInput")[:]
    text = nc.dram_tensor("text", [BL, TD], f32, kind="ExternalInput")[:]
    w_patch = nc.dram_tensor("w_patch", [H, PD], f32, kind="ExternalInput")[:]
    b_patch = nc.dram_tensor("b_patch", [H], f32, kind="ExternalInput")[:]
    w_text = nc.dram_tensor("w_text", [H, TD], f32, kind="ExternalInput")[:]
    b_text = nc.dram_tensor("b_text", [H], f32, kind="ExternalInput")[:]
    scores = nc.dram_tensor("scores", [BL, P, NB], f32, kind="ExternalOutput")[:]

    with TileContext(nc) as tc:
        with (
            tc.tile_pool(name="const", bufs=1) as const,
            tc.tile_pool(name="patch", bufs=PATCH_BUFS) as ppool,
            tc.tile_pool(name="psum", bufs=1, space=bass.MemorySpace.PSUM) as psum,
        ):
            # ---- small-weight loads ----
            # W_text natural tiles, transposed on the TensorEngine below
            # (strided DMA loads of W_text^T cost ~38us of descriptor issue).
            wt_sb = []
            for c in range(HC):
                t_ = const.tile([P, TD], f32, name=f"wt{c}")
                nc.sync.dma_start(
                    out=t_[:], in_=w_text.rearrange("(c p) td -> c p td", p=P)[c]
                )
                wt_sb.append(t_)
            txT = []
            for c in range(TC):
                t_ = const.tile([P, BL], f32, name=f"txT{c}")
                nc.sync.dma_start(
                    out=t_[:], in_=text.rearrange("b (c p) -> c p b", p=P)[c]
                )
                txT.append(t_)
            bt_row = const.tile([1, H], f32, name="bt_row")
            nc.sync.dma_start(out=bt_row[:], in_=b_text[None, :])
            bp_sb = const.tile([P, HC], f32, name="bp_sb")
            nc.sync.dma_start(out=bp_sb[:], in_=b_patch.rearrange("(c p) -> p c", p=P))
            wp_sb = []
            for c in range(HC):
                t_ = const.tile([P, PD], f32, name=f"wp{c}")
                nc.sync.dma_start(
                    out=t_[:], in_=w_patch.rearrange("(c p) d -> c p d", p=P)[c]
                )
                wp_sb.append(t_)
            ones4 = const.tile([1, BL], f32, name="ones4")
            nc.vector.memset(ones4[:], 1.0)
            ones128 = const.tile([1, P], f32, name="ones128")
            nc.vector.memset(ones128[:], 1.0)

            # ---- W_text^T via PE transpose (24 x [128,128] blocks) ----
            from concourse.masks import make_identity

            ident = const.tile([P, P], f32, name="ident")
            make_identity(nc, ident[:])
            wtT = [const.tile([P, H], f32, name=f"wtT{k}") for k in range(TC)]
            for c in range(HC):
                for k in range(TC):
                    tr_ps = psum.tile([P, P], f32, name=f"tr{c}_{k}", tag="tr", bufs=2)
                    nc.tensor.transpose(
                        tr_ps[:], wt_sb[c][:, k * P : (k + 1) * P], ident[:]
                    )
                    nc.scalar.copy(out=wtT[k][:, c * P : (c + 1) * P], in_=tr_ps[:])

            # ---- t^T[h, b] per h-chunk (PE, b_text folded via K=1 matmul) ----
            tT_sb = []
            for c in range(HC):
                tT_ps = psum.tile([P, BL], f32, name=f"tT_ps{c}", tag="tT_ps")
                for k in range(TC):
                    nc.tensor.matmul(
                        tT_ps[:],
                        lhsT=wtT[k][:, c * P : (c + 1) * P],
                        rhs=txT[k][:],
                        start=(k == 0),
                        stop=False,
                    )
                nc.tensor.matmul(
                    tT_ps[:],
                    lhsT=bt_row[0:1, c * P : (c + 1) * P],
                    rhs=ones4[:],
                    start=False,
                    stop=True,
                )
                t_ = const.tile([P, BL], f32, name=f"tT{c}")
                nc.scalar.copy(out=t_[:], in_=tT_ps[:])
                tT_sb.append(t_)

            # ---- per-batch v rows + partition-broadcast (PE + ACT) ----
            vbc = []
            for b in range(BL):
                v_row = const.tile([1, PD], f32, name=f"v_row{b}")
                for half in range(PD // 512):
                    v_ps = psum.tile([1, 512], f32, name=f"v_ps{b}_{half}", tag="v_ps")
                    for c in range(HC):
                        nc.tensor.matmul(
                            v_ps[:],
                            lhsT=tT_sb[c][:, b : b + 1],
                            rhs=wp_sb[c][:, half * 512 : (half + 1) * 512],
                            start=(c == 0),
                            stop=(c == HC - 1),
                        )
                    nc.scalar.copy(
                        out=v_row[0:1, half * 512 : (half + 1) * 512], in_=v_ps[:]
                    )
                vb_sb = const.tile([P, PD], f32, name=f"vbc{b}")
                for half in range(PD // 512):
                    vb_ps = psum.tile(
                        [P, 512], f32, name=f"vb_ps{b}_{half}", tag="vb_ps", bufs=2
                    )
                    nc.tensor.matmul(
                        vb_ps[:],
                        lhsT=ones128[:],
                        rhs=v_row[0:1, half * 512 : (half + 1) * 512],
                        start=True,
                        stop=True,
                    )
                    nc.scalar.copy(
                        out=vb_sb[:, half * 512 : (half + 1) * 512], in_=vb_ps[:]
                    )
                vbc.append(vb_sb)

            # ---- per-batch bias rows + broadcast ----
            br_sb = const.tile([1, BL], f32, name="br_sb")
            for b in range(BL):
                br_ps = psum.tile([1, 1], f32, name=f"br_ps{b}", tag="br_ps")
                for c in range(HC):
                    nc.tensor.matmul(
                        br_ps[:],
                        lhsT=tT_sb[c][:, b : b + 1],
                        rhs=bp_sb[:, c : c + 1],
                        start=(c == 0),
                        stop=(c == HC - 1),
                    )
                nc.scalar.copy(out=br_sb[0:1, b : b + 1], in_=br_ps[:])
            bbc_ps = psum.tile([P, BL], f32, name="bbc_ps", tag="bbc_ps")
            nc.tensor.matmul(
                bbc_ps[:], lhsT=ones128[:], rhs=br_sb[:], start=True, stop=True
            )
            bbc = const.tile([P, BL], f32, name="bbc")
            nc.scalar.copy(out=bbc[:], in_=bbc_ps[:])

            # ---- main loop: scores[b, n] = bias[b] + patches[b, n, :] . v[b] ----
            prod = const.tile([P, PD], f32, name="prod")
            for b in range(BL):
                sc_sb = const.tile([P, NB], f32, name=f"sc{b}")
                pr = patches[b].rearrange("(t j p) d -> t p j d", p=P, j=JPT)
                for t in range(NT):
                    tile_ = ppool.tile([P, JPT, PD], f32, tag="ptile", name="ptile")
                    dma_eng = nc.sync if (b * NT + t) % 2 == 0 else nc.scalar
                    dma_eng.dma_start(out=tile_[:], in_=pr[t])
                    for j in range(JPT):
                        nc.vector.scalar_tensor_tensor(
                            out=prod[:],
                            in0=tile_[:, j, :],
                            scalar=1.0,
                            in1=vbc[b][:, :],
                            op0=mult,
                            op1=mult,
                            accum_out=sc_sb[:, t * JPT + j : t * JPT + j + 1],
                        )
                nc.vector.tensor_scalar_add(
                    out=sc_sb[:, :], in0=sc_sb[:, :], scalar1=bbc[:, b : b + 1]
                )
                nc.sync.dma_start(out=scores[b], in_=sc_sb[:])

    nc.compile()
    return nc


def _get_nc():
    if "nc" not in _NC_CACHE:
        _NC_CACHE["nc"] = _build_nc()
    return _NC_CACHE["nc"]


def _install_profile_shim():
    """Provide antenv.axon_hooks (NTFF profiling over axon) when absent.

    Replicates trn_agent_boot's ctypes hook against libaxon_pjrt.so so
    run_bass_kernel_spmd(trace=True) can capture device profiles."""
    import contextlib
    import ctypes
    import types

    try:
        from antenv.axon_hooks import get_axon_ntff_profile_hook  # noqa: F401
        return
    except ImportError:
        pass

    so_path = "/opt/axon/libaxon_pjrt.so"
    hook = None
    if os.path.exists(so_path):
        lib = ctypes.CDLL(so_path)
        if hasattr(lib, "axon_start_nrt_profile"):
            lib.axon_start_nrt_profile.argtypes = [
                ctypes.POINTER(ctypes.c_int64),
                ctypes.c_size_t,
            ]
            lib.axon_start_nrt_profile.restype = ctypes.c_int64
            lib.axon_stop_nrt_profile.argtypes = [ctypes.c_char_p]
            lib.axon_stop_nrt_profile.restype = ctypes.c_int64

            @contextlib.contextmanager
            def _hook(output_dir, device_ids):
                import jax

                jax.devices()
                if device_ids:
                    ids = (ctypes.c_int64 * len(device_ids))(*device_ids)
                    rc = lib.axon_start_nrt_profile(ids, len(device_ids))
                else:
                    rc = lib.axon_start_nrt_profile(None, 0)
                if rc != 0:
                    raise RuntimeError(f"axon_start_nrt_profile rc={rc}")
                try:
                    yield
                finally:
                    n = lib.axon_stop_nrt_profile(str(output_dir).encode())
                    print(f"ntff profile: {n} file(s) -> {output_dir}",
                          file=sys.stderr)

            hook = _hook

    mod = types.ModuleType("antenv.axon_hooks")
    mod.get_axon_ntff_profile_hook = lambda: hook
    mod.set_axon_ntff_profile_hook = lambda h: None
    sys.modules["antenv.axon_hooks"] = mod


def kernel(**inputs):
    from concourse.bass_utils import run_bass_kernel_spmd

    global LAST_RESULTS

    patches = np.ascontiguousarray(np.asarray(inputs["patches"], dtype=np.float32))
    text = np.ascontiguousarray(np.asarray(inputs["text"], dtype=np.float32))
    w_patch = np.ascontiguousarray(np.asarray(inputs["W_patch"], dtype=np.float32))
    b_patch = np.ascontiguousarray(np.asarray(inputs["b_patch"], dtype=np.float32))
    w_text = np.ascontiguousarray(np.asarray(inputs["W_text"], dtype=np.float32))
    b_text = np.ascontiguousarray(np.asarray(inputs["b_text"], dtype=np.float32))

    nc = _get_nc()
    in_maps = []
    for c in range(NCORES):
        in_maps.append(
            {
                "patches": patches[c * BL : (c + 1) * BL],
                "text": text[c * BL : (c + 1) * BL],
                "w_patch": w_patch,
                "b_patch": b_patch,
                "w_text": w_text,
                "b_text": b_text,
            }
        )

    trace = bool(int(os.environ.get("KERNEL_PROFILE", "0")))
    if trace:
        _install_profile_shim()
        import concourse.bass_utils as _bu

        _bu.upload_artifacts = lambda tmpdir: ""  # no artifact bucket here
    res = run_bass_kernel_spmd(
        nc, in_maps, core_ids=list(range(NCORES)), trace=trace
    )
    LAST_RESULTS = res

    out = np.concatenate(
        [
            np.transpose(res.results[c]["scores"], (0, 2, 1)).reshape(BL, N)
            for c in range(NCORES)
        ],
        axis=0,
    )
    return out
